# revision 2
# baseline (speedup 1.0000x reference)
# ListFold loss (exponential transform, beta=1) on 8 Trainium2 NeuronCores.
#
# Math: with sp = pred sorted by target descending, the reference computes
#   loss = sum_i log(den_i) - (sp[i] - sp[n-1-i]),  i in [0, n/2)
#   den_i = (cp[n-i]-cp[i]) * (cm[n-i]-cm[i]) - (n-2i)
# where cp/cm are prefix sums of exp(+-sp). Re-indexing from the middle
# outward with t = n/2-1-i, u[t] = sp[n/2-1-t], v[t] = sp[n/2+t]:
#   s_plus(t)  = cumsum_incl(exp(u)+exp(v))[t]      (= cp[n-i]-cp[i])
#   s_minus(t) = cumsum_incl(exp(-u)+exp(-v))[t]
#   loss = sum_t log(s_plus*s_minus - (2t+2)) - (u[t]-v[t])
# This avoids differencing large prefix sums (exact window sums, no
# cancellation) and needs only two scan streams. The log_num part enters
# through row sums only: sum_t (u-v) is accumulated, never materialized.
#
# Sharding: the pair index t is split into 8 contiguous blocks, one per
# core, laid out [128 partitions x 4096] partition-major. Each core scans
# its block along the free axis (tensor_tensor_scan), resolves the
# partition-axis carry with a strict-triangular matmul, and the
# cross-core carry with one [8,2] AllReduce of per-block totals
# (scan-style carry exchange). Per-core partial losses are summed on the
# host (the unshard step). The argsort itself is int bookkeeping done on
# the host while sharding (XLA cannot sort on trn2 at all).

import numpy as np

N = 8388608
H = N // 2          # pairs
NCORES = 8
B = H // NCORES     # pairs per core
P = 128
C = B // P          # 4096 free-dim columns
F = 1024            # phase chunk width
NCHUNK = C // F

_CACHE = {}


def _build_nc():
    import concourse.bacc as bacc
    import concourse.mybir as mybir
    import concourse.tile as tile

    dt = mybir.dt
    f32 = dt.float32
    Alu = mybir.AluOpType
    Act = mybir.ActivationFunctionType

    nc = bacc.Bacc("TRN2", target_bir_lowering=False, debug=False,
                   num_devices=NCORES)

    u_in = nc.dram_tensor("u_in", [P, C], f32, kind="ExternalInput").ap()
    v_in = nc.dram_tensor("v_in", [P, C], f32, kind="ExternalInput").ap()
    hot8 = nc.dram_tensor("hot8", [1, NCORES], f32, kind="ExternalInput").ap()
    mask8 = nc.dram_tensor("mask8", [NCORES, 1], f32, kind="ExternalInput").ap()
    strict = nc.dram_tensor("strict", [P, P], f32, kind="ExternalInput").ap()
    ones_col = nc.dram_tensor("ones_col", [P, 1], f32, kind="ExternalInput").ap()
    ones_row = nc.dram_tensor("ones_row", [1, P], f32, kind="ExternalInput").ap()
    neg_lbase = nc.dram_tensor("neg_lbase", [P, 1], f32, kind="ExternalInput").ap()
    out_part = nc.dram_tensor("partial", [1, 1], f32, kind="ExternalOutput").ap()

    with tile.TileContext(nc) as tc:
        with (
            tc.tile_pool(name="const", bufs=1) as constp,
            tc.tile_pool(name="big", bufs=1) as bigp,
            tc.tile_pool(name="work", bufs=2) as workp,
            tc.tile_pool(name="small", bufs=2) as smallp,
            tc.tile_pool(name="acc", bufs=1) as accp,
            tc.tile_pool(name="psum", bufs=1, space="PSUM") as psump,
            tc.tile_pool(name="dram", bufs=1, space="DRAM") as dramp,
        ):
            strict_t = constp.tile([P, P], f32, tag="strict")
            ones_col_t = constp.tile([P, 1], f32, tag="ones_col")
            ones_row_t = constp.tile([1, P], f32, tag="ones_row")
            hot8_t = constp.tile([1, NCORES], f32, tag="hot8")
            mask8_t = constp.tile([NCORES, 1], f32, tag="mask8")
            neg_lbase_t = constp.tile([P, 1], f32, tag="neg_lbase")
            nc.sync.dma_start(strict_t[:], strict)
            nc.sync.dma_start(ones_col_t[:], ones_col)
            nc.sync.dma_start(ones_row_t[:], ones_row)
            nc.sync.dma_start(hot8_t[:], hot8)
            nc.sync.dma_start(mask8_t[:], mask8)
            nc.sync.dma_start(neg_lbase_t[:], neg_lbase)

            # L(t_local) = 2*(p*C + c) + 2 as f32 (exact: even ints < 2^24)
            iota_t = bigp.tile([P, C], f32, tag="iota")
            nc.gpsimd.iota(iota_t[:], pattern=[[2, C]], base=2,
                           channel_multiplier=2 * C,
                           allow_small_or_imprecise_dtypes=True)

            wp_t = bigp.tile([P, C], f32, tag="wp")   # exp(u)+exp(v)
            wm_t = bigp.tile([P, C], f32, tag="wm")   # exp(-u)+exp(-v)
            sp_t = bigp.tile([P, C], f32, tag="sp")   # scan of wp (+carry)
            sm_t = bigp.tile([P, C], f32, tag="sm")   # scan of wm (+carry)

            awp = accp.tile([P, NCHUNK], f32, tag="awp")  # row sums of wp
            awm = accp.tile([P, NCHUNK], f32, tag="awm")
            ad = accp.tile([P, NCHUNK], f32, tag="ad")    # row sums of u-v
            aln = accp.tile([P, NCHUNK], f32, tag="aln")  # row sums of ln

            # ---- phase A: exps, pair sums, row totals ----
            for c in range(NCHUNK):
                cs = slice(c * F, (c + 1) * F)
                u_t = workp.tile([P, F], f32, tag="u")
                v_t = workp.tile([P, F], f32, tag="v")
                nc.sync.dma_start(u_t[:], u_in[:, cs])
                nc.sync.dma_start(v_t[:], v_in[:, cs])

                eu = workp.tile([P, F], f32, tag="eu")
                ev = workp.tile([P, F], f32, tag="ev")
                emu = workp.tile([P, F], f32, tag="emu")
                emv = workp.tile([P, F], f32, tag="emv")
                nc.scalar.activation(eu[:], u_t[:], Act.Exp)
                nc.scalar.activation(ev[:], v_t[:], Act.Exp)
                nc.scalar.activation(emu[:], u_t[:], Act.Exp, scale=-1.0)
                nc.scalar.activation(emv[:], v_t[:], Act.Exp, scale=-1.0)

                # d scratch: only its row-sum (accum_out) is used
                d_o = workp.tile([P, F], f32, tag="dscratch")
                nc.vector.scalar_tensor_tensor(
                    out=d_o[:], in0=u_t[:], scalar=0.0, in1=v_t[:],
                    op0=Alu.add, op1=Alu.subtract, accum_out=ad[:, c:c + 1])

                nc.vector.scalar_tensor_tensor(
                    out=wp_t[:, cs], in0=eu[:], scalar=0.0, in1=ev[:],
                    op0=Alu.add, op1=Alu.add, accum_out=awp[:, c:c + 1])
                nc.vector.scalar_tensor_tensor(
                    out=wm_t[:, cs], in0=emu[:], scalar=0.0, in1=emv[:],
                    op0=Alu.add, op1=Alu.add, accum_out=awm[:, c:c + 1])

            rtp = smallp.tile([P, 1], f32, tag="rtp")
            rtm = smallp.tile([P, 1], f32, tag="rtm")
            nc.vector.tensor_reduce(rtp[:], awp[:], axis=mybir.AxisListType.X,
                                    op=Alu.add)
            nc.vector.tensor_reduce(rtm[:], awm[:], axis=mybir.AxisListType.X,
                                    op=Alu.add)

            # ---- carry exchange: block totals -> AllReduce -> offsets ----
            tot_ps = psump.tile([1, 2], f32, tag="tot")
            nc.tensor.matmul(tot_ps[:, 0:1], ones_col_t[:], rtp[:], start=True, stop=True)
            nc.tensor.matmul(tot_ps[:, 1:2], ones_col_t[:], rtm[:], start=True, stop=True)
            tot_sb = smallp.tile([1, 2], f32, tag="tot_sb")
            nc.scalar.copy(tot_sb[:], tot_ps[:])

            contrib_ps = psump.tile([NCORES, 2], f32, tag="contrib")
            nc.tensor.matmul(contrib_ps[:], hot8_t[:], tot_sb[:], start=True, stop=True)
            contrib_sb = smallp.tile([NCORES, 2], f32, tag="contrib_sb")
            nc.scalar.copy(contrib_sb[:], contrib_ps[:])

            cc_in = dramp.tile([NCORES, 2], f32, tag="cc_in")
            cc_out = dramp.tile([NCORES, 2], f32, tag="cc_out")
            nc.sync.dma_start(cc_in[:], contrib_sb[:])
            nc.gpsimd.collective_compute(
                "AllReduce", Alu.add,
                replica_groups=[list(range(NCORES))],
                ins=[cc_in.opt()], outs=[cc_out.opt()])
            allt = smallp.tile([NCORES, 2], f32, tag="allt")
            nc.sync.dma_start(allt[:], cc_out[:])

            off_ps = psump.tile([1, 2], f32, tag="off")
            nc.tensor.matmul(off_ps[:], mask8_t[:], allt[:], start=True, stop=True)
            off_sb = smallp.tile([1, 2], f32, tag="off_sb")
            nc.scalar.copy(off_sb[:], off_ps[:])

            # per-partition carries: strict partition-prefix + core offset
            carry_ps = psump.tile([P, 2], f32, tag="carry")
            nc.tensor.matmul(carry_ps[:, 0:1], strict_t[:], rtp[:], start=True, stop=False)
            nc.tensor.matmul(carry_ps[:, 0:1], ones_row_t[:], off_sb[:, 0:1], start=False, stop=True)
            nc.tensor.matmul(carry_ps[:, 1:2], strict_t[:], rtm[:], start=True, stop=False)
            nc.tensor.matmul(carry_ps[:, 1:2], ones_row_t[:], off_sb[:, 1:2], start=False, stop=True)
            carry_sb = smallp.tile([P, 2], f32, tag="carry_sb")
            nc.scalar.copy(carry_sb[:], carry_ps[:])

            # ---- scans (carry folded into the initial state) ----
            nc.vector.tensor_tensor_scan(
                sp_t[:], wp_t[:], wp_t[:], carry_sb[:, 0:1], Alu.add, Alu.bypass)
            nc.vector.tensor_tensor_scan(
                sm_t[:], wm_t[:], wm_t[:], carry_sb[:, 1:2], Alu.add, Alu.bypass)

            # ---- phase B: den, log (accumulated), final reduce ----
            for c in range(NCHUNK):
                cs = slice(c * F, (c + 1) * F)
                prod = workp.tile([P, F], f32, tag="prod")
                nc.vector.tensor_mul(prod[:], sp_t[:, cs], sm_t[:, cs])
                pi = workp.tile([P, F], f32, tag="pi")
                nc.vector.tensor_sub(pi[:], prod[:], iota_t[:, cs])
                ln_o = workp.tile([P, F], f32, tag="lnscratch")
                nc.scalar.activation(ln_o[:], pi[:], Act.Ln,
                                     bias=neg_lbase_t[:],
                                     accum_out=aln[:, c:c + 1])

            rll = smallp.tile([P, 1], f32, tag="rll")
            rld = smallp.tile([P, 1], f32, tag="rld")
            nc.vector.tensor_reduce(rll[:], aln[:], axis=mybir.AxisListType.X,
                                    op=Alu.add)
            nc.vector.tensor_reduce(rld[:], ad[:], axis=mybir.AxisListType.X,
                                    op=Alu.add)
            rowloss = smallp.tile([P, 1], f32, tag="rowloss")
            nc.vector.tensor_sub(rowloss[:], rll[:], rld[:])

            part_ps = psump.tile([1, 1], f32, tag="part")
            nc.tensor.matmul(part_ps[:], ones_col_t[:], rowloss[:], start=True, stop=True)
            part_sb = smallp.tile([1, 1], f32, tag="part_sb")
            nc.scalar.copy(part_sb[:], part_ps[:])
            nc.sync.dma_start(out_part, part_sb[:])

    nc.compile()
    return nc


def _get_nc():
    if "nc" not in _CACHE:
        _CACHE["nc"] = _build_nc()
    return _CACHE["nc"]


def _make_in_maps(pred, target):
    pred = np.ascontiguousarray(np.asarray(pred, dtype=np.float32))
    target = np.ascontiguousarray(np.asarray(target, dtype=np.float32))
    assert pred.shape == (N,) and target.shape == (N,)

    order = np.argsort(-target, kind="stable")  # matches jnp stable argsort
    sp = pred[order]
    u = sp[H - 1:: -1]  # sp[H-1-t]
    v = sp[H:]          # sp[H+t]

    strict = np.triu(np.ones((P, P), np.float32), 1)  # [k,p]=1 iff k<p
    ones_col = np.ones((P, 1), np.float32)
    ones_row = np.ones((1, P), np.float32)

    in_maps = []
    for k in range(NCORES):
        hot = np.zeros((1, NCORES), np.float32)
        hot[0, k] = 1.0
        mask = np.zeros((NCORES, 1), np.float32)
        mask[:k, 0] = 1.0
        in_maps.append({
            "u_in": np.ascontiguousarray(u[k * B:(k + 1) * B].reshape(P, C)),
            "v_in": np.ascontiguousarray(v[k * B:(k + 1) * B].reshape(P, C)),
            "hot8": hot,
            "mask8": mask,
            "strict": strict,
            "ones_col": ones_col,
            "ones_row": ones_row,
            "neg_lbase": np.full((P, 1), -2.0 * k * B, np.float32),
        })
    return in_maps


def _run(in_maps, trace=False):
    from concourse import bass_utils
    return bass_utils.run_bass_kernel_spmd(
        _get_nc(), in_maps, list(range(NCORES)), trace=trace
    )


def kernel(pred, target):
    res = _run(_make_in_maps(pred, target))
    partials = [r["partial"].reshape(()) for r in res.results]
    loss = np.float32(np.sum(np.asarray(partials, dtype=np.float64)))
    return np.asarray(loss, dtype=np.float32).reshape(())


def kernel_traced(pred, target):
    res = _run(_make_in_maps(pred, target), trace=True)
    partials = [r["partial"].reshape(()) for r in res.results]
    loss = np.float32(np.sum(np.asarray(partials, dtype=np.float64)))
    return np.asarray(loss, dtype=np.float32).reshape(()), res


# revision 6
# speedup vs baseline: 1.0343x; 1.0343x over previous
# ListFold loss (exponential transform, beta=1) on 8 Trainium2 NeuronCores.
#
# Math: with sp = pred sorted by target descending, the reference computes
#   loss = sum_i log(den_i) - (sp[i] - sp[n-1-i]),  i in [0, n/2)
#   den_i = (cp[n-i]-cp[i]) * (cm[n-i]-cm[i]) - (n-2i)
# where cp/cm are prefix sums of exp(+-sp). Re-indexing from the middle
# outward with t = n/2-1-i, u[t] = sp[n/2-1-t], v[t] = sp[n/2+t]:
#   s_plus(t)  = cumsum_incl(exp(u)+exp(v))[t]      (= cp[n-i]-cp[i])
#   s_minus(t) = cumsum_incl(exp(-u)+exp(-v))[t]
#   loss = sum_t log(s_plus*s_minus - (2t+2)) - (u[t]-v[t])
# This avoids differencing large prefix sums (exact window sums, no
# cancellation) and needs only two scan streams. The log_num part enters
# through row sums only: sum_t (u-v) is accumulated, never materialized.
#
# Sharding: the pair index t is split into 8 contiguous blocks, one per
# core, laid out [128 partitions x 4096] partition-major. Each core scans
# its block along the free axis (tensor_tensor_scan), resolves the
# partition-axis carry with a strict-triangular matmul, and the
# cross-core carry with one [8,2] AllReduce of per-block totals
# (scan-style carry exchange). Per-core partial losses are summed on the
# host (the unshard step). The argsort itself is int bookkeeping done on
# the host while sharding (XLA cannot sort on trn2 at all).

import numpy as np

N = 8388608
H = N // 2          # pairs
NCORES = 8
B = H // NCORES     # pairs per core
P = 128
C = B // P          # 4096 free-dim columns
F = 1024            # phase chunk width
NCHUNK = C // F

_CACHE = {}


def _build_nc():
    import concourse.bacc as bacc
    import concourse.mybir as mybir
    import concourse.tile as tile

    dt = mybir.dt
    f32 = dt.float32
    Alu = mybir.AluOpType
    Act = mybir.ActivationFunctionType

    nc = bacc.Bacc("TRN2", target_bir_lowering=False, debug=False,
                   num_devices=NCORES)

    u_in = nc.dram_tensor("u_in", [P, C], f32, kind="ExternalInput").ap()
    v_in = nc.dram_tensor("v_in", [P, C], f32, kind="ExternalInput").ap()
    hot8 = nc.dram_tensor("hot8", [1, NCORES], f32, kind="ExternalInput").ap()
    mask8 = nc.dram_tensor("mask8", [NCORES, 1], f32, kind="ExternalInput").ap()
    strict = nc.dram_tensor("strict", [P, P], f32, kind="ExternalInput").ap()
    ones_col = nc.dram_tensor("ones_col", [P, 1], f32, kind="ExternalInput").ap()
    ones_row = nc.dram_tensor("ones_row", [1, P], f32, kind="ExternalInput").ap()
    neg_lbase = nc.dram_tensor("neg_lbase", [P, 1], f32, kind="ExternalInput").ap()
    out_part = nc.dram_tensor("partial", [1, 1], f32, kind="ExternalOutput").ap()

    with tile.TileContext(nc) as tc:
        with (
            tc.tile_pool(name="const", bufs=1) as constp,
            tc.tile_pool(name="big", bufs=1) as bigp,
            tc.tile_pool(name="work", bufs=2) as workp,
            tc.tile_pool(name="small", bufs=2) as smallp,
            tc.tile_pool(name="acc", bufs=1) as accp,
            tc.tile_pool(name="psum", bufs=1, space="PSUM") as psump,
            tc.tile_pool(name="dram", bufs=1, space="DRAM") as dramp,
        ):
            # warm up the CC cores so the real carry AllReduce is fast
            warm_sb = constp.tile([NCORES, 2], f32, tag="warm_sb")
            nc.gpsimd.memset(warm_sb[:], 0.0)
            warm_in = dramp.tile([NCORES, 2], f32, tag="warm_in")
            warm_out = dramp.tile([NCORES, 2], f32, tag="warm_out")
            nc.sync.dma_start(warm_in[:], warm_sb[:])
            nc.gpsimd.collective_compute(
                "AllReduce", Alu.add,
                replica_groups=[list(range(NCORES))],
                ins=[warm_in.opt()], outs=[warm_out.opt()])

            strict_t = constp.tile([P, P], f32, tag="strict")
            ones_col_t = constp.tile([P, 1], f32, tag="ones_col")
            ones_row_t = constp.tile([1, P], f32, tag="ones_row")
            hot8_t = constp.tile([1, NCORES], f32, tag="hot8")
            mask8_t = constp.tile([NCORES, 1], f32, tag="mask8")
            neg_lbase_t = constp.tile([P, 1], f32, tag="neg_lbase")

            # L(t_local) = 2*(p*C + c) + 2 as f32 (exact: even ints < 2^24)
            iota_t = bigp.tile([P, C], f32, tag="iota")
            nc.gpsimd.iota(iota_t[:], pattern=[[2, C]], base=2,
                           channel_multiplier=2 * C,
                           allow_small_or_imprecise_dtypes=True)

            wp_t = bigp.tile([P, C], f32, tag="wp")   # exp(u)+exp(v)
            wm_t = bigp.tile([P, C], f32, tag="wm")   # exp(-u)+exp(-v)
            sp_t = bigp.tile([P, C], f32, tag="sp")   # scan of wp (+carry)
            sm_t = bigp.tile([P, C], f32, tag="sm")   # scan of wm (+carry)

            awp = accp.tile([P, NCHUNK], f32, tag="awp")  # row sums of wp
            awm = accp.tile([P, NCHUNK], f32, tag="awm")
            ad = accp.tile([P, NCHUNK], f32, tag="ad")    # row sums of u-v
            aln = accp.tile([P, NCHUNK], f32, tag="aln")  # row sums of ln

            # ---- phase A: exps, pair sums, row totals ----
            for c in range(NCHUNK):
                cs = slice(c * F, (c + 1) * F)
                u_t = workp.tile([P, F], f32, tag="u")
                v_t = workp.tile([P, F], f32, tag="v")
                nc.sync.dma_start(u_t[:], u_in[:, cs])
                nc.sync.dma_start(v_t[:], v_in[:, cs])

                eu = workp.tile([P, F], f32, tag="eu")
                ev = workp.tile([P, F], f32, tag="ev")
                emu = workp.tile([P, F], f32, tag="emu")
                emv = workp.tile([P, F], f32, tag="emv")
                nc.scalar.activation(eu[:], u_t[:], Act.Exp)
                nc.scalar.activation(ev[:], v_t[:], Act.Exp)
                nc.scalar.activation(emu[:], u_t[:], Act.Exp, scale=-1.0)
                nc.scalar.activation(emv[:], v_t[:], Act.Exp, scale=-1.0)

                # d scratch: only its row-sum (accum_out) is used
                d_o = workp.tile([P, F], f32, tag="dscratch")
                nc.vector.scalar_tensor_tensor(
                    out=d_o[:], in0=u_t[:], scalar=0.0, in1=v_t[:],
                    op0=Alu.add, op1=Alu.subtract, accum_out=ad[:, c:c + 1])

                nc.vector.scalar_tensor_tensor(
                    out=wp_t[:, cs], in0=eu[:], scalar=0.0, in1=ev[:],
                    op0=Alu.add, op1=Alu.add, accum_out=awp[:, c:c + 1])
                nc.vector.scalar_tensor_tensor(
                    out=wm_t[:, cs], in0=emu[:], scalar=0.0, in1=emv[:],
                    op0=Alu.add, op1=Alu.add, accum_out=awm[:, c:c + 1])

            # consts are only needed from the carry stage on — issue their
            # DMAs after the phase-A loads so chunk 0 starts sooner
            nc.sync.dma_start(strict_t[:], strict)
            nc.sync.dma_start(ones_col_t[:], ones_col)
            nc.sync.dma_start(ones_row_t[:], ones_row)
            nc.sync.dma_start(hot8_t[:], hot8)
            nc.sync.dma_start(mask8_t[:], mask8)
            nc.sync.dma_start(neg_lbase_t[:], neg_lbase)

            rtp = smallp.tile([P, 1], f32, tag="rtp")
            rtm = smallp.tile([P, 1], f32, tag="rtm")
            nc.vector.tensor_reduce(rtp[:], awp[:], axis=mybir.AxisListType.X,
                                    op=Alu.add)
            nc.vector.tensor_reduce(rtm[:], awm[:], axis=mybir.AxisListType.X,
                                    op=Alu.add)

            # ---- carry exchange: block totals -> AllReduce -> offsets ----
            tot_ps = psump.tile([1, 2], f32, tag="tot")
            nc.tensor.matmul(tot_ps[:, 0:1], ones_col_t[:], rtp[:], start=True, stop=True)
            nc.tensor.matmul(tot_ps[:, 1:2], ones_col_t[:], rtm[:], start=True, stop=True)
            tot_sb = smallp.tile([1, 2], f32, tag="tot_sb")
            nc.scalar.copy(tot_sb[:], tot_ps[:])

            contrib_ps = psump.tile([NCORES, 2], f32, tag="contrib")
            nc.tensor.matmul(contrib_ps[:], hot8_t[:], tot_sb[:], start=True, stop=True)
            contrib_sb = smallp.tile([NCORES, 2], f32, tag="contrib_sb")
            nc.scalar.copy(contrib_sb[:], contrib_ps[:])

            cc_in = dramp.tile([NCORES, 2], f32, tag="cc_in")
            cc_out = dramp.tile([NCORES, 2], f32, tag="cc_out")
            nc.sync.dma_start(cc_in[:], contrib_sb[:])
            nc.gpsimd.collective_compute(
                "AllReduce", Alu.add,
                replica_groups=[list(range(NCORES))],
                ins=[cc_in.opt()], outs=[cc_out.opt()])
            allt = smallp.tile([NCORES, 2], f32, tag="allt")
            nc.sync.dma_start(allt[:], cc_out[:])

            off_ps = psump.tile([1, 2], f32, tag="off")
            nc.tensor.matmul(off_ps[:], mask8_t[:], allt[:], start=True, stop=True)
            off_sb = smallp.tile([1, 2], f32, tag="off_sb")
            nc.scalar.copy(off_sb[:], off_ps[:])

            # per-partition carries: strict partition-prefix + core offset
            carry_ps = psump.tile([P, 2], f32, tag="carry")
            nc.tensor.matmul(carry_ps[:, 0:1], strict_t[:], rtp[:], start=True, stop=False)
            nc.tensor.matmul(carry_ps[:, 0:1], ones_row_t[:], off_sb[:, 0:1], start=False, stop=True)
            nc.tensor.matmul(carry_ps[:, 1:2], strict_t[:], rtm[:], start=True, stop=False)
            nc.tensor.matmul(carry_ps[:, 1:2], ones_row_t[:], off_sb[:, 1:2], start=False, stop=True)
            carry_sb = smallp.tile([P, 2], f32, tag="carry_sb")
            nc.scalar.copy(carry_sb[:], carry_ps[:])

            # ---- local scans (initial=0): overlap the AllReduce window ----
            nc.vector.tensor_tensor_scan(
                sp_t[:], wp_t[:], wp_t[:], 0.0, Alu.add, Alu.bypass)
            nc.vector.tensor_tensor_scan(
                sm_t[:], wm_t[:], wm_t[:], 0.0, Alu.add, Alu.bypass)

            # ---- phase B: carries folded in, den, log, final reduce ----
            for c in range(NCHUNK):
                cs = slice(c * F, (c + 1) * F)
                smf = workp.tile([P, F], f32, tag="smf")
                nc.vector.tensor_scalar_add(smf[:], sm_t[:, cs],
                                            carry_sb[:, 1:2])
                prod = workp.tile([P, F], f32, tag="prod")
                nc.vector.scalar_tensor_tensor(
                    out=prod[:], in0=sp_t[:, cs], scalar=carry_sb[:, 0:1],
                    in1=smf[:], op0=Alu.add, op1=Alu.mult)
                pi = workp.tile([P, F], f32, tag="pi")
                nc.vector.tensor_sub(pi[:], prod[:], iota_t[:, cs])
                ln_o = workp.tile([P, F], f32, tag="lnscratch")
                nc.scalar.activation(ln_o[:], pi[:], Act.Ln,
                                     bias=neg_lbase_t[:],
                                     accum_out=aln[:, c:c + 1])

            rll = smallp.tile([P, 1], f32, tag="rll")
            rld = smallp.tile([P, 1], f32, tag="rld")
            nc.vector.tensor_reduce(rll[:], aln[:], axis=mybir.AxisListType.X,
                                    op=Alu.add)
            nc.vector.tensor_reduce(rld[:], ad[:], axis=mybir.AxisListType.X,
                                    op=Alu.add)
            rowloss = smallp.tile([P, 1], f32, tag="rowloss")
            nc.vector.tensor_sub(rowloss[:], rll[:], rld[:])

            part_ps = psump.tile([1, 1], f32, tag="part")
            nc.tensor.matmul(part_ps[:], ones_col_t[:], rowloss[:], start=True, stop=True)
            part_sb = smallp.tile([1, 1], f32, tag="part_sb")
            nc.scalar.copy(part_sb[:], part_ps[:])
            nc.sync.dma_start(out_part, part_sb[:])

    nc.compile()
    return nc


def _get_nc():
    if "nc" not in _CACHE:
        _CACHE["nc"] = _build_nc()
    return _CACHE["nc"]


def _make_in_maps(pred, target):
    pred = np.ascontiguousarray(np.asarray(pred, dtype=np.float32))
    target = np.ascontiguousarray(np.asarray(target, dtype=np.float32))
    assert pred.shape == (N,) and target.shape == (N,)

    order = np.argsort(-target, kind="stable")  # matches jnp stable argsort
    sp = pred[order]
    u = sp[H - 1:: -1]  # sp[H-1-t]
    v = sp[H:]          # sp[H+t]

    strict = np.triu(np.ones((P, P), np.float32), 1)  # [k,p]=1 iff k<p
    ones_col = np.ones((P, 1), np.float32)
    ones_row = np.ones((1, P), np.float32)

    in_maps = []
    for k in range(NCORES):
        hot = np.zeros((1, NCORES), np.float32)
        hot[0, k] = 1.0
        mask = np.zeros((NCORES, 1), np.float32)
        mask[:k, 0] = 1.0
        in_maps.append({
            "u_in": np.ascontiguousarray(u[k * B:(k + 1) * B].reshape(P, C)),
            "v_in": np.ascontiguousarray(v[k * B:(k + 1) * B].reshape(P, C)),
            "hot8": hot,
            "mask8": mask,
            "strict": strict,
            "ones_col": ones_col,
            "ones_row": ones_row,
            "neg_lbase": np.full((P, 1), -2.0 * k * B, np.float32),
        })
    return in_maps


def _run(in_maps, trace=False):
    from concourse import bass_utils
    return bass_utils.run_bass_kernel_spmd(
        _get_nc(), in_maps, list(range(NCORES)), trace=trace
    )


def kernel(pred, target):
    res = _run(_make_in_maps(pred, target))
    partials = [r["partial"].reshape(()) for r in res.results]
    loss = np.float32(np.sum(np.asarray(partials, dtype=np.float64)))
    return np.asarray(loss, dtype=np.float32).reshape(())


def kernel_traced(pred, target):
    res = _run(_make_in_maps(pred, target), trace=True)
    partials = [r["partial"].reshape(()) for r in res.results]
    loss = np.float32(np.sum(np.asarray(partials, dtype=np.float64)))
    return np.asarray(loss, dtype=np.float32).reshape(()), res


# revision 8
# speedup vs baseline: 1.3779x; 1.3322x over previous
# ListFold loss (exponential transform, beta=1) on 8 Trainium2 NeuronCores.
#
# Math: with sp = pred sorted by target descending, the reference computes
#   loss = sum_i log(den_i) - (sp[i] - sp[n-1-i]),  i in [0, n/2)
#   den_i = (cp[n-i]-cp[i]) * (cm[n-i]-cm[i]) - (n-2i)
# where cp/cm are prefix sums of exp(+-sp). Re-indexing from the middle
# outward with t = n/2-1-i, u[t] = sp[n/2-1-t], v[t] = sp[n/2+t]:
#   s_plus(t)  = cumsum_incl(exp(u)+exp(v))[t]      (= cp[n-i]-cp[i])
#   s_minus(t) = cumsum_incl(exp(-u)+exp(-v))[t]
#   loss = sum_t log(s_plus*s_minus - (2t+2)) - (u[t]-v[t])
# This avoids differencing large prefix sums (exact window sums, no
# cancellation) and needs only two scan streams. The log_num part enters
# through row sums only: sum_t (u-v) is accumulated, never materialized.
#
# Sharding: the pair index t is split into 8 contiguous blocks, one per
# core, laid out [128 partitions x 4096] partition-major. Each core scans
# its block along the free axis (tensor_tensor_scan), resolves the
# partition-axis carry with a strict-triangular matmul, and the
# cross-core carry with one [8,2] AllReduce of per-block totals
# (scan-style carry exchange). Per-core partial losses are summed on the
# host (the unshard step). The argsort itself is int bookkeeping done on
# the host while sharding (XLA cannot sort on trn2 at all).

import numpy as np

N = 8388608
H = N // 2          # pairs
NCORES = 8
B = H // NCORES     # pairs per core
P = 128
C = B // P          # 4096 free-dim columns
F = 1024            # phase chunk width
NCHUNK = C // F

_CACHE = {}


def _build_nc():
    import concourse.bacc as bacc
    import concourse.mybir as mybir
    import concourse.tile as tile

    dt = mybir.dt
    f32 = dt.float32
    Alu = mybir.AluOpType
    Act = mybir.ActivationFunctionType

    nc = bacc.Bacc("TRN2", target_bir_lowering=False, debug=False,
                   num_devices=NCORES)

    u_in = nc.dram_tensor("u_in", [P, C], f32, kind="ExternalInput").ap()
    v_in = nc.dram_tensor("v_in", [P, C], f32, kind="ExternalInput").ap()
    hot8 = nc.dram_tensor("hot8", [1, NCORES], f32, kind="ExternalInput").ap()
    mask8 = nc.dram_tensor("mask8", [NCORES, 1], f32, kind="ExternalInput").ap()
    strict = nc.dram_tensor("strict", [P, P], f32, kind="ExternalInput").ap()
    ones_col = nc.dram_tensor("ones_col", [P, 1], f32, kind="ExternalInput").ap()
    ones_row = nc.dram_tensor("ones_row", [1, P], f32, kind="ExternalInput").ap()
    neg_lbase = nc.dram_tensor("neg_lbase", [P, 1], f32, kind="ExternalInput").ap()
    out_part = nc.dram_tensor("partial", [1, 1], f32, kind="ExternalOutput").ap()

    with tile.TileContext(nc) as tc:
        with (
            tc.tile_pool(name="const", bufs=1) as constp,
            tc.tile_pool(name="big", bufs=1) as bigp,
            tc.tile_pool(name="work", bufs=2) as workp,
            tc.tile_pool(name="small", bufs=2) as smallp,
            tc.tile_pool(name="acc", bufs=1) as accp,
            tc.tile_pool(name="psum", bufs=1, space="PSUM") as psump,
            tc.tile_pool(name="dram", bufs=1, space="DRAM") as dramp,
        ):
            strict_t = constp.tile([P, P], f32, tag="strict")
            ones_col_t = constp.tile([P, 1], f32, tag="ones_col")
            ones_row_t = constp.tile([1, P], f32, tag="ones_row")
            hot8_t = constp.tile([1, NCORES], f32, tag="hot8")
            mask8_t = constp.tile([NCORES, 1], f32, tag="mask8")
            neg_lbase_t = constp.tile([P, 1], f32, tag="neg_lbase")

            # L(t_local) = 2*(p*C + c) + 2 as f32 (exact: even ints < 2^24)
            iota_t = bigp.tile([P, C], f32, tag="iota")
            nc.gpsimd.iota(iota_t[:], pattern=[[2, C]], base=2,
                           channel_multiplier=2 * C,
                           allow_small_or_imprecise_dtypes=True)

            wp_t = bigp.tile([P, C], f32, tag="wp")   # exp(u)+exp(v)
            wm_t = bigp.tile([P, C], f32, tag="wm")   # exp(-u)+exp(-v)
            sp_t = bigp.tile([P, C], f32, tag="sp")   # scan of wp (+carry)
            sm_t = bigp.tile([P, C], f32, tag="sm")   # scan of wm (+carry)

            awp = accp.tile([P, NCHUNK], f32, tag="awp")  # row sums of wp
            awm = accp.tile([P, NCHUNK], f32, tag="awm")
            ad = accp.tile([P, NCHUNK], f32, tag="ad")    # row sums of u-v
            aln = accp.tile([P, NCHUNK], f32, tag="aln")  # row sums of ln

            # ---- phase A: exps, pair sums, row totals ----
            for c in range(NCHUNK):
                cs = slice(c * F, (c + 1) * F)
                u_t = workp.tile([P, F], f32, tag="u")
                v_t = workp.tile([P, F], f32, tag="v")
                nc.sync.dma_start(u_t[:], u_in[:, cs])
                nc.sync.dma_start(v_t[:], v_in[:, cs])

                eu = workp.tile([P, F], f32, tag="eu")
                ev = workp.tile([P, F], f32, tag="ev")
                emu = workp.tile([P, F], f32, tag="emu")
                emv = workp.tile([P, F], f32, tag="emv")
                nc.scalar.activation(eu[:], u_t[:], Act.Exp)
                nc.scalar.activation(ev[:], v_t[:], Act.Exp)
                nc.scalar.activation(emu[:], u_t[:], Act.Exp, scale=-1.0)
                nc.scalar.activation(emv[:], v_t[:], Act.Exp, scale=-1.0)

                # d scratch: only its row-sum (accum_out) is used
                d_o = workp.tile([P, F], f32, tag="dscratch")
                nc.vector.scalar_tensor_tensor(
                    out=d_o[:], in0=u_t[:], scalar=0.0, in1=v_t[:],
                    op0=Alu.add, op1=Alu.subtract, accum_out=ad[:, c:c + 1])

                nc.vector.scalar_tensor_tensor(
                    out=wp_t[:, cs], in0=eu[:], scalar=0.0, in1=ev[:],
                    op0=Alu.add, op1=Alu.add, accum_out=awp[:, c:c + 1])
                nc.vector.scalar_tensor_tensor(
                    out=wm_t[:, cs], in0=emu[:], scalar=0.0, in1=emv[:],
                    op0=Alu.add, op1=Alu.add, accum_out=awm[:, c:c + 1])

            # consts are only needed from the carry stage on — issue their
            # DMAs after the phase-A loads so chunk 0 starts sooner
            nc.sync.dma_start(strict_t[:], strict)
            nc.sync.dma_start(ones_col_t[:], ones_col)
            nc.sync.dma_start(ones_row_t[:], ones_row)
            nc.sync.dma_start(hot8_t[:], hot8)
            nc.sync.dma_start(mask8_t[:], mask8)
            nc.sync.dma_start(neg_lbase_t[:], neg_lbase)

            rtp = smallp.tile([P, 1], f32, tag="rtp")
            rtm = smallp.tile([P, 1], f32, tag="rtm")
            nc.vector.tensor_reduce(rtp[:], awp[:], axis=mybir.AxisListType.X,
                                    op=Alu.add)
            nc.vector.tensor_reduce(rtm[:], awm[:], axis=mybir.AxisListType.X,
                                    op=Alu.add)

            # ---- carry exchange: block totals -> AllReduce -> offsets ----
            tot_ps = psump.tile([1, 2], f32, tag="tot")
            nc.tensor.matmul(tot_ps[:, 0:1], ones_col_t[:], rtp[:], start=True, stop=True)
            nc.tensor.matmul(tot_ps[:, 1:2], ones_col_t[:], rtm[:], start=True, stop=True)
            tot_sb = smallp.tile([1, 2], f32, tag="tot_sb")
            nc.scalar.copy(tot_sb[:], tot_ps[:])

            contrib_ps = psump.tile([NCORES, 2], f32, tag="contrib")
            nc.tensor.matmul(contrib_ps[:], hot8_t[:], tot_sb[:], start=True, stop=True)
            contrib_sb = smallp.tile([NCORES, 2], f32, tag="contrib_sb")
            nc.scalar.copy(contrib_sb[:], contrib_ps[:])

            cc_in = dramp.tile([NCORES, 2], f32, tag="cc_in")
            cc_out = dramp.tile([NCORES, 2], f32, tag="cc_out")
            nc.sync.dma_start(cc_in[:], contrib_sb[:])
            nc.gpsimd.collective_compute(
                "AllReduce", Alu.add,
                replica_groups=[list(range(NCORES))],
                ins=[cc_in.opt()], outs=[cc_out.opt()])
            allt = smallp.tile([NCORES, 2], f32, tag="allt")
            nc.sync.dma_start(allt[:], cc_out[:])

            off_ps = psump.tile([1, 2], f32, tag="off")
            nc.tensor.matmul(off_ps[:], mask8_t[:], allt[:], start=True, stop=True)
            off_sb = smallp.tile([1, 2], f32, tag="off_sb")
            nc.scalar.copy(off_sb[:], off_ps[:])

            # per-partition carries: strict partition-prefix + core offset
            carry_ps = psump.tile([P, 2], f32, tag="carry")
            nc.tensor.matmul(carry_ps[:, 0:1], strict_t[:], rtp[:], start=True, stop=False)
            nc.tensor.matmul(carry_ps[:, 0:1], ones_row_t[:], off_sb[:, 0:1], start=False, stop=True)
            nc.tensor.matmul(carry_ps[:, 1:2], strict_t[:], rtm[:], start=True, stop=False)
            nc.tensor.matmul(carry_ps[:, 1:2], ones_row_t[:], off_sb[:, 1:2], start=False, stop=True)
            carry_sb = smallp.tile([P, 2], f32, tag="carry_sb")
            nc.scalar.copy(carry_sb[:], carry_ps[:])

            # ---- local scans (initial=0): overlap the AllReduce window ----
            nc.vector.tensor_tensor_scan(
                sp_t[:], wp_t[:], wp_t[:], 0.0, Alu.add, Alu.bypass)
            nc.vector.tensor_tensor_scan(
                sm_t[:], wm_t[:], wm_t[:], 0.0, Alu.add, Alu.bypass)

            # X1 = sp0*sm0 - iota, carry-independent: also runs inside the
            # AllReduce window. den = X1 + cp*sm0 + cm*sp0 + (cp*cm - lbase)
            x1_t = bigp.tile([P, C], f32, tag="x1")
            for c in range(NCHUNK):
                cs = slice(c * F, (c + 1) * F)
                prod = workp.tile([P, F], f32, tag="prod")
                nc.vector.tensor_mul(prod[:], sp_t[:, cs], sm_t[:, cs])
                nc.vector.tensor_sub(x1_t[:, cs], prod[:], iota_t[:, cs])

            # bias = cp*cm - lbase (per-partition scalars)
            cpcm = smallp.tile([P, 1], f32, tag="cpcm")
            nc.vector.tensor_mul(cpcm[:], carry_sb[:, 0:1], carry_sb[:, 1:2])
            bias_t = smallp.tile([P, 1], f32, tag="bias_t")
            nc.vector.tensor_add(bias_t[:], cpcm[:], neg_lbase_t[:])

            # ---- phase B (post-AllReduce): two fused passes + log ----
            for c in range(NCHUNK):
                cs = slice(c * F, (c + 1) * F)
                t1 = workp.tile([P, F], f32, tag="t1")
                nc.vector.scalar_tensor_tensor(
                    out=t1[:], in0=sm_t[:, cs], scalar=carry_sb[:, 0:1],
                    in1=x1_t[:, cs], op0=Alu.mult, op1=Alu.add)
                t2 = workp.tile([P, F], f32, tag="t2")
                nc.vector.scalar_tensor_tensor(
                    out=t2[:], in0=sp_t[:, cs], scalar=carry_sb[:, 1:2],
                    in1=t1[:], op0=Alu.mult, op1=Alu.add)
                ln_o = workp.tile([P, F], f32, tag="lnscratch")
                nc.scalar.activation(ln_o[:], t2[:], Act.Ln,
                                     bias=bias_t[:],
                                     accum_out=aln[:, c:c + 1])

            rll = smallp.tile([P, 1], f32, tag="rll")
            rld = smallp.tile([P, 1], f32, tag="rld")
            nc.vector.tensor_reduce(rll[:], aln[:], axis=mybir.AxisListType.X,
                                    op=Alu.add)
            nc.vector.tensor_reduce(rld[:], ad[:], axis=mybir.AxisListType.X,
                                    op=Alu.add)
            rowloss = smallp.tile([P, 1], f32, tag="rowloss")
            nc.vector.tensor_sub(rowloss[:], rll[:], rld[:])

            part_ps = psump.tile([1, 1], f32, tag="part")
            nc.tensor.matmul(part_ps[:], ones_col_t[:], rowloss[:], start=True, stop=True)
            part_sb = smallp.tile([1, 1], f32, tag="part_sb")
            nc.scalar.copy(part_sb[:], part_ps[:])
            nc.sync.dma_start(out_part, part_sb[:])

    nc.compile()
    return nc


def _get_nc():
    if "nc" not in _CACHE:
        _CACHE["nc"] = _build_nc()
    return _CACHE["nc"]


def _make_in_maps(pred, target):
    pred = np.ascontiguousarray(np.asarray(pred, dtype=np.float32))
    target = np.ascontiguousarray(np.asarray(target, dtype=np.float32))
    assert pred.shape == (N,) and target.shape == (N,)

    order = np.argsort(-target, kind="stable")  # matches jnp stable argsort
    sp = pred[order]
    u = sp[H - 1:: -1]  # sp[H-1-t]
    v = sp[H:]          # sp[H+t]

    strict = np.triu(np.ones((P, P), np.float32), 1)  # [k,p]=1 iff k<p
    ones_col = np.ones((P, 1), np.float32)
    ones_row = np.ones((1, P), np.float32)

    in_maps = []
    for k in range(NCORES):
        hot = np.zeros((1, NCORES), np.float32)
        hot[0, k] = 1.0
        mask = np.zeros((NCORES, 1), np.float32)
        mask[:k, 0] = 1.0
        in_maps.append({
            "u_in": np.ascontiguousarray(u[k * B:(k + 1) * B].reshape(P, C)),
            "v_in": np.ascontiguousarray(v[k * B:(k + 1) * B].reshape(P, C)),
            "hot8": hot,
            "mask8": mask,
            "strict": strict,
            "ones_col": ones_col,
            "ones_row": ones_row,
            "neg_lbase": np.full((P, 1), -2.0 * k * B, np.float32),
        })
    return in_maps


def _run(in_maps, trace=False):
    from concourse import bass_utils
    return bass_utils.run_bass_kernel_spmd(
        _get_nc(), in_maps, list(range(NCORES)), trace=trace
    )


def kernel(pred, target):
    res = _run(_make_in_maps(pred, target))
    partials = [r["partial"].reshape(()) for r in res.results]
    loss = np.float32(np.sum(np.asarray(partials, dtype=np.float64)))
    return np.asarray(loss, dtype=np.float32).reshape(())


def kernel_traced(pred, target):
    res = _run(_make_in_maps(pred, target), trace=True)
    partials = [r["partial"].reshape(()) for r in res.results]
    loss = np.float32(np.sum(np.asarray(partials, dtype=np.float64)))
    return np.asarray(loss, dtype=np.float32).reshape(()), res


# revision 9
# speedup vs baseline: 1.3941x; 1.0118x over previous
# ListFold loss (exponential transform, beta=1) on 8 Trainium2 NeuronCores.
#
# Math: with sp = pred sorted by target descending, the reference computes
#   loss = sum_i log(den_i) - (sp[i] - sp[n-1-i]),  i in [0, n/2)
#   den_i = (cp[n-i]-cp[i]) * (cm[n-i]-cm[i]) - (n-2i)
# where cp/cm are prefix sums of exp(+-sp). Re-indexing from the middle
# outward with t = n/2-1-i, u[t] = sp[n/2-1-t], v[t] = sp[n/2+t]:
#   s_plus(t)  = cumsum_incl(exp(u)+exp(v))[t]      (= cp[n-i]-cp[i])
#   s_minus(t) = cumsum_incl(exp(-u)+exp(-v))[t]
#   loss = sum_t log(s_plus*s_minus - (2t+2)) - (u[t]-v[t])
# This avoids differencing large prefix sums (exact window sums, no
# cancellation) and needs only two scan streams. The log_num part enters
# through row sums only: sum_t (u-v) is accumulated, never materialized.
#
# Sharding: the pair index t is split into 8 contiguous blocks, one per
# core, laid out [128 partitions x 4096] partition-major. Each core scans
# its block along the free axis (tensor_tensor_scan), resolves the
# partition-axis carry with a strict-triangular matmul, and the
# cross-core carry with one [8,2] AllReduce of per-block totals
# (scan-style carry exchange). Per-core partial losses are summed on the
# host (the unshard step). The argsort itself is int bookkeeping done on
# the host while sharding (XLA cannot sort on trn2 at all).

import numpy as np

N = 8388608
H = N // 2          # pairs
NCORES = 8
B = H // NCORES     # pairs per core
P = 128
C = B // P          # 4096 free-dim columns
F = 1024            # phase chunk width
NCHUNK = C // F

_CACHE = {}


def _build_nc():
    import concourse.bacc as bacc
    import concourse.mybir as mybir
    import concourse.tile as tile

    dt = mybir.dt
    f32 = dt.float32
    Alu = mybir.AluOpType
    Act = mybir.ActivationFunctionType

    nc = bacc.Bacc("TRN2", target_bir_lowering=False, debug=False,
                   num_devices=NCORES)

    u_in = nc.dram_tensor("u_in", [P, C], f32, kind="ExternalInput").ap()
    v_in = nc.dram_tensor("v_in", [P, C], f32, kind="ExternalInput").ap()
    hot8 = nc.dram_tensor("hot8", [1, NCORES], f32, kind="ExternalInput").ap()
    maskbc = nc.dram_tensor("maskbc", [NCORES, P], f32, kind="ExternalInput").ap()
    strict = nc.dram_tensor("strict", [P, P], f32, kind="ExternalInput").ap()
    ones_col = nc.dram_tensor("ones_col", [P, 1], f32, kind="ExternalInput").ap()
    ones_row = nc.dram_tensor("ones_row", [1, P], f32, kind="ExternalInput").ap()
    neg_lbase = nc.dram_tensor("neg_lbase", [P, 1], f32, kind="ExternalInput").ap()
    out_part = nc.dram_tensor("partial", [1, 1], f32, kind="ExternalOutput").ap()

    with tile.TileContext(nc) as tc:
        with (
            tc.tile_pool(name="const", bufs=1) as constp,
            tc.tile_pool(name="big", bufs=1) as bigp,
            tc.tile_pool(name="work", bufs=2) as workp,
            tc.tile_pool(name="small", bufs=2) as smallp,
            tc.tile_pool(name="acc", bufs=1) as accp,
            tc.tile_pool(name="psum", bufs=1, space="PSUM") as psump,
            tc.tile_pool(name="dram", bufs=1, space="DRAM") as dramp,
        ):
            strict_t = constp.tile([P, P], f32, tag="strict")
            ones_col_t = constp.tile([P, 1], f32, tag="ones_col")
            ones_row_t = constp.tile([1, P], f32, tag="ones_row")
            hot8_t = constp.tile([1, NCORES], f32, tag="hot8")
            maskbc_t = constp.tile([NCORES, P], f32, tag="maskbc")
            neg_lbase_t = constp.tile([P, 1], f32, tag="neg_lbase")

            # L(t_local) = 2*(p*C + c) + 2 as f32 (exact: even ints < 2^24)
            iota_t = bigp.tile([P, C], f32, tag="iota")
            nc.gpsimd.iota(iota_t[:], pattern=[[2, C]], base=2,
                           channel_multiplier=2 * C,
                           allow_small_or_imprecise_dtypes=True)

            wp_t = bigp.tile([P, C], f32, tag="wp")   # exp(u)+exp(v)
            wm_t = bigp.tile([P, C], f32, tag="wm")   # exp(-u)+exp(-v)
            sp_t = bigp.tile([P, C], f32, tag="sp")   # scan of wp (+carry)
            sm_t = bigp.tile([P, C], f32, tag="sm")   # scan of wm (+carry)

            awp = accp.tile([P, NCHUNK], f32, tag="awp")  # row sums of wp
            awm = accp.tile([P, NCHUNK], f32, tag="awm")
            ad = accp.tile([P, NCHUNK], f32, tag="ad")    # row sums of u-v
            aln = accp.tile([P, NCHUNK], f32, tag="aln")  # row sums of ln

            # ---- phase A: exps, pair sums, row totals ----
            for c in range(NCHUNK):
                cs = slice(c * F, (c + 1) * F)
                u_t = workp.tile([P, F], f32, tag="u")
                v_t = workp.tile([P, F], f32, tag="v")
                nc.sync.dma_start(u_t[:], u_in[:, cs])
                nc.sync.dma_start(v_t[:], v_in[:, cs])

                eu = workp.tile([P, F], f32, tag="eu")
                ev = workp.tile([P, F], f32, tag="ev")
                emu = workp.tile([P, F], f32, tag="emu")
                emv = workp.tile([P, F], f32, tag="emv")
                nc.scalar.activation(eu[:], u_t[:], Act.Exp)
                nc.scalar.activation(ev[:], v_t[:], Act.Exp)
                nc.scalar.activation(emu[:], u_t[:], Act.Exp, scale=-1.0)
                nc.scalar.activation(emv[:], v_t[:], Act.Exp, scale=-1.0)

                # d scratch: only its row-sum (accum_out) is used
                d_o = workp.tile([P, F], f32, tag="dscratch")
                nc.vector.scalar_tensor_tensor(
                    out=d_o[:], in0=u_t[:], scalar=0.0, in1=v_t[:],
                    op0=Alu.add, op1=Alu.subtract, accum_out=ad[:, c:c + 1])

                nc.vector.scalar_tensor_tensor(
                    out=wp_t[:, cs], in0=eu[:], scalar=0.0, in1=ev[:],
                    op0=Alu.add, op1=Alu.add, accum_out=awp[:, c:c + 1])
                nc.vector.scalar_tensor_tensor(
                    out=wm_t[:, cs], in0=emu[:], scalar=0.0, in1=emv[:],
                    op0=Alu.add, op1=Alu.add, accum_out=awm[:, c:c + 1])

            # consts are only needed from the carry stage on — issue their
            # DMAs after the phase-A loads so chunk 0 starts sooner
            nc.sync.dma_start(strict_t[:], strict)
            nc.sync.dma_start(ones_col_t[:], ones_col)
            nc.sync.dma_start(ones_row_t[:], ones_row)
            nc.sync.dma_start(hot8_t[:], hot8)
            nc.sync.dma_start(maskbc_t[:], maskbc)
            nc.sync.dma_start(neg_lbase_t[:], neg_lbase)

            rld = smallp.tile([P, 1], f32, tag="rld")
            nc.vector.tensor_reduce(rld[:], ad[:], axis=mybir.AxisListType.X,
                                    op=Alu.add)

            rtp = smallp.tile([P, 1], f32, tag="rtp")
            rtm = smallp.tile([P, 1], f32, tag="rtm")
            nc.vector.tensor_reduce(rtp[:], awp[:], axis=mybir.AxisListType.X,
                                    op=Alu.add)
            nc.vector.tensor_reduce(rtm[:], awm[:], axis=mybir.AxisListType.X,
                                    op=Alu.add)

            # ---- carry exchange: block totals -> AllReduce -> offsets ----
            tot_ps = psump.tile([1, 2], f32, tag="tot")
            nc.tensor.matmul(tot_ps[:, 0:1], ones_col_t[:], rtp[:], start=True, stop=True)
            nc.tensor.matmul(tot_ps[:, 1:2], ones_col_t[:], rtm[:], start=True, stop=True)
            tot_sb = smallp.tile([1, 2], f32, tag="tot_sb")
            nc.scalar.copy(tot_sb[:], tot_ps[:])

            contrib_ps = psump.tile([NCORES, 2], f32, tag="contrib")
            nc.tensor.matmul(contrib_ps[:], hot8_t[:], tot_sb[:], start=True, stop=True)
            contrib_sb = smallp.tile([NCORES, 2], f32, tag="contrib_sb")
            nc.scalar.copy(contrib_sb[:], contrib_ps[:])

            cc_in = dramp.tile([NCORES, 2], f32, tag="cc_in")
            cc_out = dramp.tile([NCORES, 2], f32, tag="cc_out")
            nc.sync.dma_start(cc_in[:], contrib_sb[:])
            nc.gpsimd.collective_compute(
                "AllReduce", Alu.add,
                replica_groups=[list(range(NCORES))],
                ins=[cc_in.opt()], outs=[cc_out.opt()])
            allt = smallp.tile([NCORES, 2], f32, tag="allt")
            nc.sync.dma_start(allt[:], cc_out[:])

            # local strict-prefix part of the carry: ready pre-AllReduce
            carry_loc_ps = psump.tile([P, 2], f32, tag="carry_loc")
            nc.tensor.matmul(carry_loc_ps[:, 0:1], strict_t[:], rtp[:], start=True, stop=True)
            nc.tensor.matmul(carry_loc_ps[:, 1:2], strict_t[:], rtm[:], start=True, stop=True)
            carry_loc_sb = smallp.tile([P, 2], f32, tag="carry_loc_sb")
            nc.scalar.copy(carry_loc_sb[:], carry_loc_ps[:])

            # post-AllReduce: one matmul broadcasts the masked core offset
            bc_ps = psump.tile([P, 2], f32, tag="bc")
            nc.tensor.matmul(bc_ps[:], maskbc_t[:], allt[:], start=True, stop=True)
            carry_sb = smallp.tile([P, 2], f32, tag="carry_sb")
            nc.vector.tensor_add(carry_sb[:], carry_loc_sb[:], bc_ps[:])

            # ---- local scans (initial=0): overlap the AllReduce window ----
            nc.vector.tensor_tensor_scan(
                sp_t[:], wp_t[:], wp_t[:], 0.0, Alu.add, Alu.bypass)
            nc.vector.tensor_tensor_scan(
                sm_t[:], wm_t[:], wm_t[:], 0.0, Alu.add, Alu.bypass)

            # X1 = sp0*sm0 - iota, carry-independent: also runs inside the
            # AllReduce window. den = X1 + cp*sm0 + cm*sp0 + (cp*cm - lbase)
            x1_t = bigp.tile([P, C], f32, tag="x1")
            for c in range(NCHUNK):
                cs = slice(c * F, (c + 1) * F)
                prod = workp.tile([P, F], f32, tag="prod")
                nc.vector.tensor_mul(prod[:], sp_t[:, cs], sm_t[:, cs])
                nc.vector.tensor_sub(x1_t[:, cs], prod[:], iota_t[:, cs])

            # warm the Ln activation table while ACT is idle
            lnwarm = smallp.tile([P, 1], f32, tag="lnwarm")
            nc.scalar.activation(lnwarm[:], awp[:, NCHUNK - 1:NCHUNK],
                                 Act.Ln)

            # bias = cp*cm - lbase (per-partition scalars)
            cpcm = smallp.tile([P, 1], f32, tag="cpcm")
            nc.vector.tensor_mul(cpcm[:], carry_sb[:, 0:1], carry_sb[:, 1:2])
            bias_t = smallp.tile([P, 1], f32, tag="bias_t")
            nc.vector.tensor_add(bias_t[:], cpcm[:], neg_lbase_t[:])

            # ---- phase B (post-AllReduce): two fused passes + log ----
            for c in range(NCHUNK):
                cs = slice(c * F, (c + 1) * F)
                t1 = workp.tile([P, F], f32, tag="t1")
                nc.vector.scalar_tensor_tensor(
                    out=t1[:], in0=sm_t[:, cs], scalar=carry_sb[:, 0:1],
                    in1=x1_t[:, cs], op0=Alu.mult, op1=Alu.add)
                t2 = workp.tile([P, F], f32, tag="t2")
                nc.vector.scalar_tensor_tensor(
                    out=t2[:], in0=sp_t[:, cs], scalar=carry_sb[:, 1:2],
                    in1=t1[:], op0=Alu.mult, op1=Alu.add)
                ln_o = workp.tile([P, F], f32, tag="lnscratch")
                nc.scalar.activation(ln_o[:], t2[:], Act.Ln,
                                     bias=bias_t[:],
                                     accum_out=aln[:, c:c + 1])

            rll = smallp.tile([P, 1], f32, tag="rll")
            nc.vector.tensor_reduce(rll[:], aln[:], axis=mybir.AxisListType.X,
                                    op=Alu.add)
            rowloss = smallp.tile([P, 1], f32, tag="rowloss")
            nc.vector.tensor_sub(rowloss[:], rll[:], rld[:])

            part_ps = psump.tile([1, 1], f32, tag="part")
            nc.tensor.matmul(part_ps[:], ones_col_t[:], rowloss[:], start=True, stop=True)
            part_sb = smallp.tile([1, 1], f32, tag="part_sb")
            nc.scalar.copy(part_sb[:], part_ps[:])
            nc.sync.dma_start(out_part, part_sb[:])

    nc.compile()
    return nc


def _get_nc():
    if "nc" not in _CACHE:
        _CACHE["nc"] = _build_nc()
    return _CACHE["nc"]


def _make_in_maps(pred, target):
    pred = np.ascontiguousarray(np.asarray(pred, dtype=np.float32))
    target = np.ascontiguousarray(np.asarray(target, dtype=np.float32))
    assert pred.shape == (N,) and target.shape == (N,)

    order = np.argsort(-target, kind="stable")  # matches jnp stable argsort
    sp = pred[order]
    u = sp[H - 1:: -1]  # sp[H-1-t]
    v = sp[H:]          # sp[H+t]

    strict = np.triu(np.ones((P, P), np.float32), 1)  # [k,p]=1 iff k<p
    ones_col = np.ones((P, 1), np.float32)
    ones_row = np.ones((1, P), np.float32)

    in_maps = []
    for k in range(NCORES):
        hot = np.zeros((1, NCORES), np.float32)
        hot[0, k] = 1.0
        mask = np.zeros((NCORES, P), np.float32)
        mask[:k, :] = 1.0
        in_maps.append({
            "u_in": np.ascontiguousarray(u[k * B:(k + 1) * B].reshape(P, C)),
            "v_in": np.ascontiguousarray(v[k * B:(k + 1) * B].reshape(P, C)),
            "hot8": hot,
            "maskbc": mask,
            "strict": strict,
            "ones_col": ones_col,
            "ones_row": ones_row,
            "neg_lbase": np.full((P, 1), -2.0 * k * B, np.float32),
        })
    return in_maps


def _run(in_maps, trace=False):
    from concourse import bass_utils
    return bass_utils.run_bass_kernel_spmd(
        _get_nc(), in_maps, list(range(NCORES)), trace=trace
    )


def kernel(pred, target):
    res = _run(_make_in_maps(pred, target))
    partials = [r["partial"].reshape(()) for r in res.results]
    loss = np.float32(np.sum(np.asarray(partials, dtype=np.float64)))
    return np.asarray(loss, dtype=np.float32).reshape(())


def kernel_traced(pred, target):
    res = _run(_make_in_maps(pred, target), trace=True)
    partials = [r["partial"].reshape(()) for r in res.results]
    loss = np.float32(np.sum(np.asarray(partials, dtype=np.float64)))
    return np.asarray(loss, dtype=np.float32).reshape(()), res
